# revision 1
# baseline (speedup 1.0000x reference)
"""Trainium2 Bass kernel for nn_BaseModel_7885559955990 (gnn_message_passing).

Model: 2 tiny GCN layers on a 1024-node graph -> flatten to v[16384] ->
relu(v @ L1_w[16384,16384] + L1_b) -> sigmoid(. @ L2_w[16384,32] + L2_b).

Distribution (8 cores, tensor-parallel per the sharding hint):
  - L1_w is sharded column-wise: core c computes v1_c = relu(v @ L1_w[:, c*2048:(c+1)*2048] + b_c)
  - L2_w is sharded row-wise:    core c computes partial_c = v1_c @ L2_w[c*2048:(c+1)*2048, :]
  - unshard = sum partials over cores, + L2_b, sigmoid  (32 floats, done host-side)
  - GCN layers are tiny and replicated on every core.

The graph operator (degree-normalized adjacency with self loops) depends only
on the edge-list input; it is densified host-side into AT[s, d] (4 MB) so the
message-passing aggregation runs as dense matmuls on the tensor engine.

The dominant cost is streaming the 128 MiB/core L1 slice from HBM
(~360 GB/s/core). The vector-matrix product uses v-chunks as the [128,1]
stationary operand so the PE streams weight columns at line rate.

Precision modes for the L1 stream (MODE):
  fp32  - exact; PE-bound (fp32 streams at 4 cyc/row): ~440 us
  f32r  - fp32 data, single-pass reduced-precision matmul: ~DMA roofline
  bf16  - bf16 weights: half the HBM traffic, ~2x faster than roofline
  split - W and v split into bf16 hi+lo pairs (3 matmul passes); same HBM
          bytes as fp32 but full-rate streaming -> DMA roofline with ~1e-6 err
"""

import numpy as np
import ml_dtypes
from contextlib import ExitStack

import concourse.bacc as bacc
import concourse.tile as tile
from concourse import mybir
from concourse.bass_utils import run_bass_kernel_spmd

F32 = mybir.dt.float32
F32R = mybir.dt.float32r
BF16 = mybir.dt.bfloat16
AF = mybir.ActivationFunctionType

N_CORES = 8
N_NODES = 1024
C = 16                    # GCN channel width
M = N_NODES * C           # 16384 flattened width
COLS = M // N_CORES       # 2048 L1 columns per core
N_OUT = 32
NK = M // 128             # 128 contraction chunks of 128

MODE = "split"            # default; see module docstring
TRACE = False             # set True (module-level) to profile; result in LAST_RESULT
LAST_RESULT = None

_MODE_CFG = {
    #        stream_dt, agg_dt, cpd (128-row chunks per DMA), split
    "fp32":  (F32,  F32,  2, False),
    "f32r":  (F32R, F32,  2, False),
    "bf16":  (BF16, BF16, 4, False),
    "split": (BF16, F32,  2, True),
}

# All DRAM tensors streamed at rate are pre-tiled on the host into
# partition-major [128, ...] layout so every dma_start is a plain 2D AP —
# 3D/rearranged APs defeat the 16-engine descriptor spray (measured
# 17 GB/s vs 287 GB/s per core).

_program_cache = {}


def _build(mode, repeat=1):
    # repeat > 1 duplicates the weight-stream phase (timing builds only):
    # wall-slope between two repeat values isolates the steady-state
    # stream+matmul rate, cancelling RPC overhead and kernel prefix/tail.
    stream_dt, agg_dt, cpd, split = _MODE_CFG[mode]
    np_stream = ml_dtypes.bfloat16 if stream_dt == BF16 else np.float32
    np_agg = ml_dtypes.bfloat16 if agg_dt == BF16 else np.float32

    nc = bacc.Bacc("TRN2", target_bir_lowering=False, debug=False,
                   num_devices=N_CORES)

    # ---- DRAM tensors (per-core views; replicated unless noted).
    # at/l1w/l2w are host-pre-tiled partition-major (see _prep_inputs).
    at = nc.dram_tensor("at", [128, 8 * N_NODES], agg_dt, kind="ExternalInput").ap()
    xt = nc.dram_tensor("xt", [C, N_NODES], F32, kind="ExternalInput").ap()
    w1 = nc.dram_tensor("w1", [C, C], F32, kind="ExternalInput").ap()
    b1 = nc.dram_tensor("b1", [C, 1], F32, kind="ExternalInput").ap()
    w2 = nc.dram_tensor("w2", [C, C], F32, kind="ExternalInput").ap()
    b2 = nc.dram_tensor("b2", [C, 1], F32, kind="ExternalInput").ap()
    sub = 2 if split else 1       # sub-chunks (hi/lo) per 128-row chunk
    # +8 KB pad per partition row: a power-of-two row stride aliases DRAM
    # banks (measured 228 -> 384 GB/s/core on the 128 MB stream)
    pad = 8192 // (2 if stream_dt == BF16 else 4)
    l1w = nc.dram_tensor("l1w", [128, NK * sub * COLS + pad], stream_dt,
                         kind="ExternalInput").ap()
    l1bt = nc.dram_tensor("l1bt", [128, COLS // 128], F32, kind="ExternalInput").ap()
    l2w = nc.dram_tensor("l2w", [128, (COLS // 128) * N_OUT], F32,
                         kind="ExternalInput").ap()
    out = nc.dram_tensor("out", [1, N_OUT], F32, kind="ExternalOutput").ap()

    n_vj = COLS // 128            # 16 v1 chunks
    n_ng = COLS // 512            # 4 psum bank groups for the big matmul
    ndma = NK // cpd              # big-stream DMA count

    with tile.TileContext(nc) as tc, ExitStack() as ctx:
        const = ctx.enter_context(tc.tile_pool(name="const", bufs=1))
        small = ctx.enter_context(tc.tile_pool(name="small", bufs=1))
        wpool = ctx.enter_context(tc.tile_pool(name="wpool", bufs=8))
        dpool = ctx.enter_context(tc.tile_pool(name="dpool", bufs=1, space="DRAM"))

        # ---- constant loads (issued first so they beat the weight stream
        # into the DMA queues)
        at_sb = const.tile([128, 8 * N_NODES], agg_dt, tag="at")
        nc.sync.dma_start(at_sb[:, :], at)
        xt_sb = const.tile([C, N_NODES], F32, tag="xt")
        nc.sync.dma_start(xt_sb[:, :], xt)
        w1_sb = const.tile([C, C], F32, tag="w1")
        nc.sync.dma_start(w1_sb[:, :], w1)
        b1_sb = const.tile([C, 1], F32, tag="b1")
        nc.sync.dma_start(b1_sb[:, :], b1)
        w2_sb = const.tile([C, C], F32, tag="w2")
        nc.sync.dma_start(w2_sb[:, :], w2)
        b2_sb = const.tile([C, 1], F32, tag="b2")
        nc.sync.dma_start(b2_sb[:, :], b2)
        l1bt_sb = const.tile([128, n_vj], F32, tag="l1bt")
        nc.sync.dma_start(l1bt_sb[:, :], l1bt)
        l2w_sb = const.tile([128, n_vj * N_OUT], F32, tag="l2w")
        nc.sync.dma_start(l2w_sb[:, :], l2w)

        # ---- GCN: two layers of  hT' = relu( (AT.T-aggregated (h W)) + b )
        # h is kept transposed: [16 channels (partitions), 1024 nodes].
        def gcn_layer(h_in, w_sb, b_sb, psz, psh, zpool, hpool, li):
            # z = h @ W, built node-tile-major: z_i [128 nodes, 16]
            z_tiles = []
            for i in range(8):
                zps = psz.tile([128, C], F32, tag="zps")
                nc.tensor.matmul(zps[:, :], h_in[:, 128 * i:128 * (i + 1)],
                                 w_sb[:, :], start=True, stop=True)
                z_sb = zpool.tile([128, C], agg_dt, tag=f"z{li}_{i}")
                nc.vector.tensor_copy(z_sb[:, :], zps[:, :])
                z_tiles.append(z_sb)
            # aggregate: outT[c, d] = sum_s z[s, c] * AT[s, d]
            hps = psh.tile([C, N_NODES], F32, tag="hps")
            for i in range(8):
                for hh in range(2):
                    nc.tensor.matmul(
                        hps[:, 512 * hh:512 * (hh + 1)],
                        z_tiles[i][:, :],
                        at_sb[:, 1024 * i + 512 * hh:1024 * i + 512 * (hh + 1)],
                        start=(i == 0), stop=(i == 7),
                    )
            h_out = hpool.tile([C, N_NODES], F32, tag=f"h{li}")
            nc.scalar.activation(h_out[:, :], hps[:, :], AF.Relu, bias=b_sb[:, :])
            return h_out

        with tc.tile_pool(name="psz", bufs=2, space="PSUM") as psz, \
             tc.tile_pool(name="psh", bufs=2, space="PSUM") as psh, \
             tc.tile_pool(name="zpool", bufs=1) as zpool, \
             tc.tile_pool(name="hpool", bufs=1) as hpool:
            h1 = gcn_layer(xt_sb, w1_sb, b1_sb, psz, psh, zpool, hpool, 1)
            h2 = gcn_layer(h1, w2_sb, b2_sb, psz, psh, zpool, hpool, 2)

            # ---- vcol: v-chunks as stationary columns. vcol[16a+c, k] = v[128k+16a+c]
            # = h2[8k+a, c] = h2T[c, 8k+a]
            vcol = small.tile([128, NK], F32, tag="vcol")
            h2v = h2[:, :].rearrange("c (k a) -> c k a", a=8)
            for a in range(8):
                nc.gpsimd.dma_start(vcol[16 * a:16 * (a + 1), :], h2v[:, :, a])

        if split:
            vhi = small.tile([128, NK], BF16, tag="vhi")
            nc.vector.tensor_copy(vhi[:, :], vcol[:, :])
            vhi_f = small.tile([128, NK], F32, tag="vhif")
            nc.vector.tensor_copy(vhi_f[:, :], vhi[:, :])
            vlo_f = small.tile([128, NK], F32, tag="vlof")
            nc.vector.tensor_sub(vlo_f[:, :], vcol[:, :], vhi_f[:, :])
            vlo = small.tile([128, NK], BF16, tag="vlo")
            nc.vector.tensor_copy(vlo[:, :], vlo_f[:, :])
            # passes: (stationary vec, hi/lo weight sub-chunk)
            passes = [(vhi, 0), (vlo, 0), (vhi, 1)]
        elif stream_dt == F32:
            passes = [(vcol, 0)]
        else:
            vs = small.tile([128, NK], stream_dt, tag="vs")
            nc.vector.tensor_copy(vs[:, :], vcol[:, :])
            passes = [(vs, 0)]

        # ---- big matmul: vps[0, n] = sum_k v[k] * L1[k, n]
        with tc.tile_pool(name="psv", bufs=1, space="PSUM") as psv, \
             tc.tile_pool(name="ps32", bufs=1, space="PSUM") as ps32:
            vps = psv.tile([1, COLS], F32, tag="vps")
            wfree = COLS * sub * cpd     # tile free elems per DMA
            for rep in range(repeat):
                for t in range(ndma):
                    wt = wpool.tile([128, wfree], stream_dt, tag="w")
                    nc.sync.dma_start(wt[:, :], l1w[:, wfree * t:wfree * (t + 1)])
                    for cc in range(cpd):
                        k = cpd * t + cc
                        for j in range(n_ng):
                            for si, (vv, wi) in enumerate(passes):
                                base = (sub * cc + wi) * 2048
                                nc.tensor.matmul(
                                    vps[0:1, 512 * j:512 * (j + 1)],
                                    vv[:, k:k + 1],
                                    wt[:, base + 512 * j:base + 512 * (j + 1)],
                                    start=(k == 0 and si == 0 and rep == 0),
                                    stop=(k == NK - 1 and si == len(passes) - 1
                                          and rep == repeat - 1),
                                )

            # ---- tail: v1 = relu(vps + b), re-laid out to [128, 16] via DRAM bounce
            v1row = small.tile([1, COLS], F32, tag="v1row")
            nc.scalar.copy(v1row[:, :], vps[0:1, :])
            dscratch = dpool.tile([1, COLS], F32, tag="dscratch")
            nc.gpsimd.dma_start(dscratch[:, :], v1row[:, :])
            v1t = small.tile([128, n_vj], F32, tag="v1t")
            nc.gpsimd.dma_start(
                v1t[:, :], dscratch[:, :].rearrange("o (j p) -> p (o j)", p=128))
            nc.vector.tensor_add(v1t[:, :], v1t[:, :], l1bt_sb[:, :])
            nc.vector.tensor_relu(v1t[:, :], v1t[:, :])

            # ---- second matmul: partial[1, 32] = sum_j v1_j^T @ L2_j
            p32 = ps32.tile([1, N_OUT], F32, tag="p32")
            for j in range(n_vj):
                nc.tensor.matmul(
                    p32[0:1, :], v1t[:, j:j + 1],
                    l2w_sb[:, N_OUT * j:N_OUT * (j + 1)],
                    start=(j == 0), stop=(j == n_vj - 1),
                )
            out_sb = small.tile([1, N_OUT], F32, tag="out")
            nc.vector.tensor_copy(out_sb[:, :], p32[0:1, :])
            nc.sync.dma_start(out, out_sb[:, :])

    nc.compile()
    return nc


def _host_adjacency(edge):
    """Dense AT[s, d] = sum over (self-looped, deg-normalized) edges s->d."""
    src = edge[0].astype(np.int64)
    dst = edge[1].astype(np.int64)
    loop = np.arange(N_NODES, dtype=np.int64)
    s = np.concatenate([src, loop])
    d = np.concatenate([dst, loop])
    deg = np.bincount(d, minlength=N_NODES).astype(np.float32)
    dinv = np.where(deg > 0, deg, np.float32(1.0)) ** np.float32(-0.5)
    norm = (dinv[s] * dinv[d]).astype(np.float32)
    at = np.zeros((N_NODES, N_NODES), np.float32)
    np.add.at(at, (s, d), norm)
    return at


def _prep_inputs(x, edge, W1, b1, W2, b2, L1_w, L1_b, L2_w, mode):
    stream_dt, agg_dt, cpd, split = _MODE_CFG[mode]
    np_stream = ml_dtypes.bfloat16 if stream_dt == BF16 else np.float32
    np_agg = ml_dtypes.bfloat16 if agg_dt == BF16 else np.float32

    # partition-major tiling: AT [1024,1024] -> [128, 8*1024] with
    # at_t[p, 1024*i + d] = AT[128*i + p, d]
    at = _host_adjacency(edge).astype(np_agg)
    at = np.ascontiguousarray(
        at.reshape(8, 128, N_NODES).transpose(1, 0, 2).reshape(128, 8 * N_NODES))
    xt = np.ascontiguousarray(np.asarray(x, np.float32).T)
    w1 = np.ascontiguousarray(np.asarray(W1, np.float32))
    b1v = np.asarray(b1, np.float32).reshape(C, 1).copy()
    w2 = np.ascontiguousarray(np.asarray(W2, np.float32))
    b2v = np.asarray(b2, np.float32).reshape(C, 1).copy()
    L1_w = np.asarray(L1_w, np.float32)
    L1_b = np.asarray(L1_b, np.float32)
    L2_w = np.asarray(L2_w, np.float32)

    in_maps = []
    for c in range(N_CORES):
        sl = slice(COLS * c, COLS * (c + 1))
        wsl = np.ascontiguousarray(L1_w[:, sl])
        pad = 8192 // (2 if np_stream == ml_dtypes.bfloat16 else 4)
        if split:
            hi = wsl.astype(ml_dtypes.bfloat16)
            lo = (wsl - hi.astype(np.float32)).astype(ml_dtypes.bfloat16)
            # partition-major, k-major then hi/lo:
            # l1[p, (2k+s)*2048 + n] = (hi if s==0 else lo)[128k+p, n]
            body = np.empty((NK, 2, 128, COLS), ml_dtypes.bfloat16)
            body[:, 0] = hi.reshape(NK, 128, COLS)
            body[:, 1] = lo.reshape(NK, 128, COLS)
            body = body.transpose(2, 0, 1, 3).reshape(128, NK * 2 * COLS)
        else:
            # l1[p, 2048k + n] = Wslice[128k + p, n]
            body = (wsl.astype(np_stream).reshape(NK, 128, COLS)
                    .transpose(1, 0, 2).reshape(128, NK * COLS))
        l1 = np.zeros((128, body.shape[1] + pad), np_stream)
        l1[:, :body.shape[1]] = body
        l1bt = np.ascontiguousarray(L1_b[sl].reshape(COLS // 128, 128).T)
        # l2[p, 32j + n] = L2slice[128j + p, n]
        l2 = np.ascontiguousarray(
            L2_w[sl, :].reshape(COLS // 128, 128, N_OUT)
            .transpose(1, 0, 2).reshape(128, (COLS // 128) * N_OUT))
        in_maps.append(dict(at=at, xt=xt, w1=w1, b1=b1v, w2=w2, b2=b2v,
                            l1w=l1, l1bt=l1bt, l2w=l2))
    return in_maps


def kernel(**inputs):
    global LAST_RESULT
    mode = MODE
    if mode not in _program_cache:
        _program_cache[mode] = _build(mode)
    nc = _program_cache[mode]

    in_maps = _prep_inputs(
        inputs["x"], inputs["edge"], inputs["W1"], inputs["b1"],
        inputs["W2"], inputs["b2"], inputs["L1_w"], inputs["L1_b"],
        inputs["L2_w"], mode)

    res = run_bass_kernel_spmd(
        nc, in_maps, core_ids=list(range(N_CORES)), trace=TRACE)
    LAST_RESULT = res

    partial = np.zeros(N_OUT, np.float64)
    for r in res.results:
        partial += r["out"].reshape(-1).astype(np.float64)
    logits = partial.astype(np.float32) + np.asarray(inputs["L2_b"], np.float32)
    return (1.0 / (1.0 + np.exp(-logits))).astype(np.float32)



# revision 11
# speedup vs baseline: 3.1373x; 3.1373x over previous
"""Trainium2 Bass kernel for nn_BaseModel_7885559955990 (gnn_message_passing).

Model: 2 tiny GCN layers on a 1024-node graph -> flatten to v[16384] ->
relu(v @ L1_w[16384,16384] + L1_b) -> sigmoid(. @ L2_w[16384,32] + L2_b).

Distribution (8 cores, tensor-parallel per the sharding hint):
  - L1_w is sharded column-wise: core c computes v1_c = relu(v @ L1_w[:, c*2048:(c+1)*2048] + b_c)
  - L2_w is sharded row-wise:    core c computes partial_c = v1_c @ L2_w[c*2048:(c+1)*2048, :]
  - unshard = sum partials over cores, + L2_b, sigmoid  (32 floats, done host-side)
  - GCN layers are tiny and replicated on every core.

The graph operator (degree-normalized adjacency with self loops) depends only
on the edge-list input; it is densified host-side into AT[s, d] (4 MB) so the
message-passing aggregation runs as dense matmuls on the tensor engine.

The dominant cost is streaming the 128 MiB/core L1 slice from HBM
(~360 GB/s/core). The vector-matrix product uses v-chunks as the [128,1]
stationary operand so the PE streams weight columns at line rate.

Precision modes for the L1 stream (MODE):
  fp32  - exact; PE-bound (fp32 streams at 4 cyc/row): ~440 us
  f32r  - fp32 data, single-pass reduced-precision matmul: ~DMA roofline
  bf16  - bf16 weights: half the HBM traffic, ~2x faster than roofline
  split - W and v split into bf16 hi+lo pairs (3 matmul passes); same HBM
          bytes as fp32 but full-rate streaming -> DMA roofline with ~1e-6 err
  fp8   - e4m3 weights (x2^10) and v (x2^4, folded into W2/b2), descale
          folded into L1_b/L2_w; DoubleRow matmuls (2 k-chunks per pass at
          0.5 cyc/row). Quarter HBM traffic. End-to-end max rel err ~3e-3
          (final logits are tiny, sigmoid amplification ~0.5, and quant
          noise sqrt-cancels over the 16384-term contraction).
"""

import numpy as np
import ml_dtypes
from contextlib import ExitStack

import concourse.bacc as bacc
import concourse.tile as tile
from concourse import mybir
from concourse.bass_utils import run_bass_kernel_spmd

F32 = mybir.dt.float32
F32R = mybir.dt.float32r
BF16 = mybir.dt.bfloat16
F8E4 = mybir.dt.float8e4
AF = mybir.ActivationFunctionType

N_CORES = 8
N_NODES = 1024
C = 16                    # GCN channel width
M = N_NODES * C           # 16384 flattened width
COLS = M // N_CORES       # 2048 L1 columns per core
N_OUT = 32
NK = M // 128             # 128 contraction chunks of 128

MODE = "fp8"              # default; see module docstring
TRACE = False             # set True (module-level) to profile; result in LAST_RESULT
LAST_RESULT = None

_MODE_CFG = {
    #        stream_dt, agg_dt, cpd (128-row chunks per DMA), split
    "fp32":  (F32,  F32,  2, False),
    "f32r":  (F32R, F32,  2, False),
    "bf16":  (BF16, BF16, 4, False),
    "split": (BF16, F32,  2, True),
    "fp8":   (F8E4, BF16, 8, False),
}

_DT_SIZE = {F32: 4, F32R: 4, BF16: 2, F8E4: 1}
# scaled-domain factors for fp8: W1x2^10, v (h2) x2^4 -> psum x2^14
W_SCALE = 2.0 ** 10
V_SCALE = 2.0 ** 4
Z_SCALE = W_SCALE * V_SCALE

# All DRAM tensors streamed at rate are pre-tiled on the host into
# partition-major [128, ...] layout so every dma_start is a plain 2D AP —
# 3D/rearranged APs defeat the 16-engine descriptor spray (measured
# 17 GB/s vs 287 GB/s per core).

_program_cache = {}


def _build(mode, repeat=1):
    # repeat > 1 duplicates the weight-stream phase (timing builds only):
    # wall-slope between two repeat values isolates the steady-state
    # stream+matmul rate, cancelling RPC overhead and kernel prefix/tail.
    stream_dt, agg_dt, cpd, split = _MODE_CFG[mode]
    dr = stream_dt == F8E4       # DoubleRow: two k-chunks per matmul pass

    nc = bacc.Bacc("TRN2", target_bir_lowering=False, debug=False,
                   num_devices=N_CORES)

    # ---- DRAM tensors (per-core views; replicated unless noted).
    # at/l1w/l2w are host-pre-tiled partition-major (see _prep_inputs).
    at = nc.dram_tensor("at", [128, 8 * N_NODES], agg_dt, kind="ExternalInput").ap()
    xt = nc.dram_tensor("xt", [C, N_NODES], F32, kind="ExternalInput").ap()
    w1 = nc.dram_tensor("w1", [C, C], F32, kind="ExternalInput").ap()
    b1 = nc.dram_tensor("b1", [C, 1], F32, kind="ExternalInput").ap()
    w2 = nc.dram_tensor("w2", [C, C], F32, kind="ExternalInput").ap()
    b2 = nc.dram_tensor("b2", [C, 1], F32, kind="ExternalInput").ap()
    sub = 2 if split else 1       # sub-chunks (hi/lo) per 128-row chunk
    # +8 KB pad per partition row: a power-of-two row stride aliases DRAM
    # banks (measured 228 -> 384 GB/s/core on the 128 MB stream)
    pad = 8192 // _DT_SIZE[stream_dt]
    l1w = nc.dram_tensor("l1w", [128, NK * sub * COLS + pad], stream_dt,
                         kind="ExternalInput").ap()
    l1bt = nc.dram_tensor("l1bt", [128, COLS // 128], F32, kind="ExternalInput").ap()
    l2w = nc.dram_tensor("l2w", [128, (COLS // 128) * N_OUT], F32,
                         kind="ExternalInput").ap()
    out = nc.dram_tensor("out", [1, N_OUT], F32, kind="ExternalOutput").ap()

    n_vj = COLS // 128            # 16 v1 chunks
    n_ng = COLS // 512            # 4 psum bank groups for the big matmul
    ndma = NK // cpd              # big-stream DMA count

    with tile.TileContext(nc) as tc, ExitStack() as ctx:
        const = ctx.enter_context(tc.tile_pool(name="const", bufs=1))
        small = ctx.enter_context(tc.tile_pool(name="small", bufs=1))
        wpool = ctx.enter_context(tc.tile_pool(name="wpool", bufs=8))
        dpool = ctx.enter_context(tc.tile_pool(name="dpool", bufs=1, space="DRAM"))

        # ---- constant loads (issued first so they beat the weight stream
        # into the DMA queues)
        at_sb = const.tile([128, 8 * N_NODES], agg_dt, tag="at")
        nc.sync.dma_start(at_sb[:, :], at)
        xt_sb = const.tile([C, N_NODES], F32, tag="xt")
        nc.sync.dma_start(xt_sb[:, :], xt)
        w1_sb = const.tile([C, C], F32, tag="w1")
        nc.sync.dma_start(w1_sb[:, :], w1)
        b1_sb = const.tile([C, 1], F32, tag="b1")
        nc.sync.dma_start(b1_sb[:, :], b1)
        w2_sb = const.tile([C, C], F32, tag="w2")
        nc.sync.dma_start(w2_sb[:, :], w2)
        b2_sb = const.tile([C, 1], F32, tag="b2")
        nc.sync.dma_start(b2_sb[:, :], b2)
        l1bt_sb = const.tile([128, n_vj], F32, tag="l1bt")
        nc.sync.dma_start(l1bt_sb[:, :], l1bt)
        l2w_sb = const.tile([128, n_vj * N_OUT], F32, tag="l2w")
        nc.sync.dma_start(l2w_sb[:, :], l2w)

        # ---- GCN: two layers of  hT' = relu( (AT.T-aggregated (h W)) + b )
        # h is kept transposed: [16 channels (partitions), 1024 nodes].
        def gcn_layer(h_in, w_sb, b_sb, psz, psh, zpool, hpool, li):
            # z = h @ W, built node-tile-major: z_i [128 nodes, 16]
            z_tiles = []
            for i in range(8):
                zps = psz.tile([128, C], F32, tag="zps")
                nc.tensor.matmul(zps[:, :], h_in[:, 128 * i:128 * (i + 1)],
                                 w_sb[:, :], start=True, stop=True)
                z_sb = zpool.tile([128, C], agg_dt, tag=f"z{li}_{i}")
                nc.vector.tensor_copy(z_sb[:, :], zps[:, :])
                z_tiles.append(z_sb)
            # aggregate: outT[c, d] = sum_s z[s, c] * AT[s, d]
            hps = psh.tile([C, N_NODES], F32, tag="hps")
            for i in range(8):
                for hh in range(2):
                    nc.tensor.matmul(
                        hps[:, 512 * hh:512 * (hh + 1)],
                        z_tiles[i][:, :],
                        at_sb[:, 1024 * i + 512 * hh:1024 * i + 512 * (hh + 1)],
                        start=(i == 0), stop=(i == 7),
                    )
            h_out = hpool.tile([C, N_NODES], F32, tag=f"h{li}")
            nc.scalar.activation(h_out[:, :], hps[:, :], AF.Relu, bias=b_sb[:, :])
            return h_out

        with tc.tile_pool(name="psz", bufs=2, space="PSUM") as psz, \
             tc.tile_pool(name="psh", bufs=2, space="PSUM") as psh, \
             tc.tile_pool(name="zpool", bufs=1) as zpool, \
             tc.tile_pool(name="hpool", bufs=1) as hpool:
            h1 = gcn_layer(xt_sb, w1_sb, b1_sb, psz, psh, zpool, hpool, 1)
            h2 = gcn_layer(h1, w2_sb, b2_sb, psz, psh, zpool, hpool, 2)

            # ---- vcol: v-chunks as stationary columns. vcol[16a+c, k] = v[128k+16a+c]
            # = h2[8k+a, c] = h2T[c, 8k+a]
            vcol = small.tile([128, NK], F32, tag="vcol")
            h2v = h2[:, :].rearrange("c (k a) -> c k a", a=8)
            for a in range(8):
                nc.gpsimd.dma_start(vcol[16 * a:16 * (a + 1), :], h2v[:, :, a])

        if split:
            vhi = small.tile([128, NK], BF16, tag="vhi")
            nc.vector.tensor_copy(vhi[:, :], vcol[:, :])
            vhi_f = small.tile([128, NK], F32, tag="vhif")
            nc.vector.tensor_copy(vhi_f[:, :], vhi[:, :])
            vlo_f = small.tile([128, NK], F32, tag="vlof")
            nc.vector.tensor_sub(vlo_f[:, :], vcol[:, :], vhi_f[:, :])
            vlo = small.tile([128, NK], BF16, tag="vlo")
            nc.vector.tensor_copy(vlo[:, :], vlo_f[:, :])
            # passes: (stationary vec, hi/lo weight sub-chunk)
            passes = [(vhi, 0), (vlo, 0), (vhi, 1)]
        elif stream_dt == F32:
            passes = [(vcol, 0)]
        elif dr:
            # DoubleRow ldweights wants a 3D AP [128, slot=2, pair] whose
            # slot step is a multiple of 16 elems -> deinterleave v chunks:
            # v8[p, s*64 + P] = v[128*(2P+s) + p]
            vs = small.tile([128, NK], stream_dt, tag="vs")
            nc.vector.tensor_copy(
                vs[:, :].rearrange("p (s q) -> p s q", s=2),
                vcol[:, :].rearrange("p (q s) -> p s q", s=2))
            passes = [(vs, 0)]
        else:
            vs = small.tile([128, NK], stream_dt, tag="vs")
            nc.vector.tensor_copy(vs[:, :], vcol[:, :])
            passes = [(vs, 0)]

        # ---- big matmul: vps[0, n] = sum_k v[k] * L1[k, n]
        with tc.tile_pool(name="psv", bufs=1, space="PSUM") as psv, \
             tc.tile_pool(name="ps32", bufs=1, space="PSUM") as ps32:
            vps = psv.tile([1, COLS], F32, tag="vps")
            wfree = COLS * sub * cpd     # tile free elems per DMA
            npair = NK // 2
            for rep in range(repeat):
                for t in range(ndma):
                    wt = wpool.tile([128, wfree], stream_dt, tag="w")
                    nc.sync.dma_start(wt[:, :], l1w[:, wfree * t:wfree * (t + 1)])
                    if dr:
                        # DoubleRow: [128, 2, 512] moving AP covers chunks
                        # (2P, 2P+1) at 0.5 cyc/row
                        v3 = passes[0][0][:, :].rearrange(
                            "p (s q) -> p s q", s=2)
                        for pp in range(cpd // 2):
                            P = (cpd // 2) * t + pp
                            w3 = wt[:, 2 * pp * COLS:(2 * pp + 2) * COLS] \
                                .rearrange("p (s c) -> p s c", s=2)
                            for j in range(n_ng):
                                nc.tensor.matmul(
                                    vps[0:1, 512 * j:512 * (j + 1)],
                                    v3[:, :, P:P + 1],
                                    w3[:, :, 512 * j:512 * (j + 1)],
                                    start=(P == 0 and rep == 0),
                                    stop=(P == npair - 1 and rep == repeat - 1),
                                    perf_mode=mybir.MatmulPerfMode.DoubleRow,
                                )
                        continue
                    for cc in range(cpd):
                        k = cpd * t + cc
                        for j in range(n_ng):
                            for si, (vv, wi) in enumerate(passes):
                                base = (sub * cc + wi) * 2048
                                nc.tensor.matmul(
                                    vps[0:1, 512 * j:512 * (j + 1)],
                                    vv[:, k:k + 1],
                                    wt[:, base + 512 * j:base + 512 * (j + 1)],
                                    start=(k == 0 and si == 0 and rep == 0),
                                    stop=(k == NK - 1 and si == len(passes) - 1
                                          and rep == repeat - 1),
                                )

            # ---- tail: v1 = relu(vps + b), re-laid out to [128, 16] via DRAM bounce
            v1row = small.tile([1, COLS], F32, tag="v1row")
            nc.scalar.copy(v1row[:, :], vps[0:1, :])
            dscratch = dpool.tile([1, COLS], F32, tag="dscratch")
            nc.gpsimd.dma_start(dscratch[:, :], v1row[:, :])
            v1t = small.tile([128, n_vj], F32, tag="v1t")
            nc.gpsimd.dma_start(
                v1t[:, :], dscratch[:, :].rearrange("o (j p) -> p (o j)", p=128))
            nc.vector.tensor_add(v1t[:, :], v1t[:, :], l1bt_sb[:, :])
            nc.vector.tensor_relu(v1t[:, :], v1t[:, :])

            # ---- second matmul: partial[1, 32] = sum_j v1_j^T @ L2_j
            p32 = ps32.tile([1, N_OUT], F32, tag="p32")
            for j in range(n_vj):
                nc.tensor.matmul(
                    p32[0:1, :], v1t[:, j:j + 1],
                    l2w_sb[:, N_OUT * j:N_OUT * (j + 1)],
                    start=(j == 0), stop=(j == n_vj - 1),
                )
            out_sb = small.tile([1, N_OUT], F32, tag="out")
            nc.vector.tensor_copy(out_sb[:, :], p32[0:1, :])
            nc.sync.dma_start(out, out_sb[:, :])

    nc.compile()
    return nc


def _host_adjacency(edge):
    """Dense AT[s, d] = sum over (self-looped, deg-normalized) edges s->d."""
    src = edge[0].astype(np.int64)
    dst = edge[1].astype(np.int64)
    loop = np.arange(N_NODES, dtype=np.int64)
    s = np.concatenate([src, loop])
    d = np.concatenate([dst, loop])
    deg = np.bincount(d, minlength=N_NODES).astype(np.float32)
    dinv = np.where(deg > 0, deg, np.float32(1.0)) ** np.float32(-0.5)
    norm = (dinv[s] * dinv[d]).astype(np.float32)
    at = np.zeros((N_NODES, N_NODES), np.float32)
    np.add.at(at, (s, d), norm)
    return at


_NP_DT = {F32: np.float32, F32R: np.float32, BF16: ml_dtypes.bfloat16,
          F8E4: ml_dtypes.float8_e4m3}


def _prep_inputs(x, edge, W1, b1, W2, b2, L1_w, L1_b, L2_w, mode):
    stream_dt, agg_dt, cpd, split = _MODE_CFG[mode]
    np_stream = _NP_DT[stream_dt]
    np_agg = _NP_DT[agg_dt]
    fp8 = stream_dt == F8E4
    # fp8 scaled domain: h2 (=v) carries x2^4 via W2/b2, W stream x2^10,
    # so psum is x2^14; descale via L1_b x2^14 and L2_w x2^-14.
    vs = V_SCALE if fp8 else 1.0
    ws = W_SCALE if fp8 else 1.0
    zs = vs * ws

    # partition-major tiling: AT [1024,1024] -> [128, 8*1024] with
    # at_t[p, 1024*i + d] = AT[128*i + p, d]
    at = _host_adjacency(edge).astype(np_agg)
    at = np.ascontiguousarray(
        at.reshape(8, 128, N_NODES).transpose(1, 0, 2).reshape(128, 8 * N_NODES))
    xt = np.ascontiguousarray(np.asarray(x, np.float32).T)
    w1 = np.ascontiguousarray(np.asarray(W1, np.float32))
    b1v = np.asarray(b1, np.float32).reshape(C, 1).copy()
    w2 = np.ascontiguousarray(np.asarray(W2, np.float32) * vs)
    b2v = (np.asarray(b2, np.float32) * vs).reshape(C, 1).copy()
    L1_w = np.asarray(L1_w, np.float32)
    L1_b = np.asarray(L1_b, np.float32) * zs
    L2_w = np.asarray(L2_w, np.float32) * (1.0 / zs)

    in_maps = []
    for c in range(N_CORES):
        sl = slice(COLS * c, COLS * (c + 1))
        wsl = np.ascontiguousarray(L1_w[:, sl]) * ws
        pad = 8192 // np.dtype(np_stream).itemsize
        if split:
            hi = wsl.astype(ml_dtypes.bfloat16)
            lo = (wsl - hi.astype(np.float32)).astype(ml_dtypes.bfloat16)
            # partition-major, k-major then hi/lo:
            # l1[p, (2k+s)*2048 + n] = (hi if s==0 else lo)[128k+p, n]
            body = np.empty((NK, 2, 128, COLS), ml_dtypes.bfloat16)
            body[:, 0] = hi.reshape(NK, 128, COLS)
            body[:, 1] = lo.reshape(NK, 128, COLS)
            body = body.transpose(2, 0, 1, 3).reshape(128, NK * 2 * COLS)
        else:
            # l1[p, 2048k + n] = Wslice[128k + p, n]
            body = (wsl.astype(np_stream).reshape(NK, 128, COLS)
                    .transpose(1, 0, 2).reshape(128, NK * COLS))
        l1 = np.zeros((128, body.shape[1] + pad), np_stream)
        l1[:, :body.shape[1]] = body
        l1bt = np.ascontiguousarray(L1_b[sl].reshape(COLS // 128, 128).T)
        # l2[p, 32j + n] = L2slice[128j + p, n]
        l2 = np.ascontiguousarray(
            L2_w[sl, :].reshape(COLS // 128, 128, N_OUT)
            .transpose(1, 0, 2).reshape(128, (COLS // 128) * N_OUT))
        in_maps.append(dict(at=at, xt=xt, w1=w1, b1=b1v, w2=w2, b2=b2v,
                            l1w=l1, l1bt=l1bt, l2w=l2))
    return in_maps


def kernel(**inputs):
    global LAST_RESULT
    mode = MODE
    if mode not in _program_cache:
        _program_cache[mode] = _build(mode)
    nc = _program_cache[mode]

    in_maps = _prep_inputs(
        inputs["x"], inputs["edge"], inputs["W1"], inputs["b1"],
        inputs["W2"], inputs["b2"], inputs["L1_w"], inputs["L1_b"],
        inputs["L2_w"], mode)

    res = run_bass_kernel_spmd(
        nc, in_maps, core_ids=list(range(N_CORES)), trace=TRACE)
    LAST_RESULT = res

    partial = np.zeros(N_OUT, np.float64)
    for r in res.results:
        partial += r["out"].reshape(-1).astype(np.float64)
    logits = partial.astype(np.float32) + np.asarray(inputs["L2_b"], np.float32)
    return (1.0 / (1.0 + np.exp(-logits))).astype(np.float32)



# revision 37
# speedup vs baseline: 4.0503x; 1.2910x over previous
"""Trainium2 Bass kernel for nn_BaseModel_7885559955990 (gnn_message_passing).

Model: 2 tiny GCN layers on a 1024-node graph -> flatten to v[16384] ->
relu(v @ L1_w[16384,16384] + L1_b) -> sigmoid(. @ L2_w[16384,32] + L2_b).

Distribution (8 cores, tensor-parallel per the sharding hint):
  - L1_w is sharded column-wise: core c computes v1_c = relu(v @ L1_w[:, c*2048:(c+1)*2048] + b_c)
  - L2_w is sharded row-wise:    core c computes partial_c = v1_c @ L2_w[c*2048:(c+1)*2048, :]
  - unshard = sum partials over cores, + L2_b, sigmoid  (32 floats, done host-side)
  - GCN layers are tiny and replicated on every core.

The graph operator (degree-normalized adjacency with self loops) depends only
on the edge-list input; it is densified host-side into AT[s, d] (4 MB) so the
message-passing aggregation runs as dense matmuls on the tensor engine.

The dominant cost is streaming the 128 MiB/core L1 slice from HBM
(~360 GB/s/core). The vector-matrix product uses v-chunks as the [128,1]
stationary operand so the PE streams weight columns at line rate.

Precision modes for the L1 stream (MODE):
  fp32  - exact; PE-bound (fp32 streams at 4 cyc/row): ~440 us
  f32r  - fp32 data, single-pass reduced-precision matmul: ~DMA roofline
  bf16  - bf16 weights: half the HBM traffic, ~2x faster than roofline
  split - W and v split into bf16 hi+lo pairs (3 matmul passes); same HBM
          bytes as fp32 but full-rate streaming -> DMA roofline with ~1e-6 err
  fp8   - e4m3 weights (x2^10) and v (x2^4, folded into W2/b2), descale
          folded into L1_b/L2_w; DoubleRow matmuls (2 k-chunks per pass at
          0.5 cyc/row). Quarter HBM traffic. End-to-end max rel err ~3e-3
          (final logits are tiny, sigmoid amplification ~0.5, and quant
          noise sqrt-cancels over the 16384-term contraction).
"""

import numpy as np
import ml_dtypes
from contextlib import ExitStack

import concourse.bacc as bacc
import concourse.tile as tile
from concourse import mybir
from concourse.bass_utils import run_bass_kernel_spmd

F32 = mybir.dt.float32
F32R = mybir.dt.float32r
BF16 = mybir.dt.bfloat16
F8E4 = mybir.dt.float8e4
AF = mybir.ActivationFunctionType

N_CORES = 8
N_NODES = 1024
C = 16                    # GCN channel width
M = N_NODES * C           # 16384 flattened width
COLS = M // N_CORES       # 2048 L1 columns per core
N_OUT = 32
NK = M // 128             # 128 contraction chunks of 128

MODE = "fp8"              # default; see module docstring
TRACE = False             # set True (module-level) to profile; result in LAST_RESULT
LAST_RESULT = None

_MODE_CFG = {
    #        stream_dt, agg_dt, cpd (128-row chunks per DMA), split
    "fp32":  (F32,  F32,  2, False),
    "f32r":  (F32R, F32,  2, False),
    "bf16":  (BF16, BF16, 4, False),
    "split": (BF16, F32,  2, True),
    "fp8":   (F8E4, BF16, 4, False),
}

_DT_SIZE = {F32: 4, F32R: 4, BF16: 2, F8E4: 1}
# scaled-domain factors for fp8: W1x2^10, v (h2) x2^4 -> psum x2^14
W_SCALE = 2.0 ** 10
V_SCALE = 2.0 ** 4
Z_SCALE = W_SCALE * V_SCALE

# All DRAM tensors streamed at rate are pre-tiled on the host into
# partition-major [128, ...] layout so every dma_start is a plain 2D AP —
# 3D/rearranged APs defeat the 16-engine descriptor spray (measured
# 17 GB/s vs 287 GB/s per core).

_program_cache = {}


def _build(mode, repeat=1):
    # repeat > 1 duplicates the weight-stream phase (timing builds only):
    # wall-slope between two repeat values isolates the steady-state
    # stream+matmul rate, cancelling RPC overhead and kernel prefix/tail.
    stream_dt, agg_dt, cpd, split = _MODE_CFG[mode]
    dr = stream_dt == F8E4       # DoubleRow: two k-chunks per matmul pass

    nc = bacc.Bacc("TRN2", target_bir_lowering=False, debug=False,
                   num_devices=N_CORES)

    # ---- DRAM tensors (per-core views; replicated unless noted).
    # at/l1w/l2w are host-pre-tiled partition-major (see _prep_inputs).
    at = nc.dram_tensor("at", [128, 8 * N_NODES], F8E4 if dr else agg_dt,
                        kind="ExternalInput").ap()
    xt = nc.dram_tensor("xt", [C, N_NODES], F32, kind="ExternalInput").ap()
    w1 = nc.dram_tensor("w1", [C, C], F32, kind="ExternalInput").ap()
    b1 = nc.dram_tensor("b1", [C, 1], F32, kind="ExternalInput").ap()
    w2 = nc.dram_tensor("w2", [C, C], F32, kind="ExternalInput").ap()
    b2 = nc.dram_tensor("b2", [C, 1], F32, kind="ExternalInput").ap()
    sub = 2 if split else 1       # sub-chunks (hi/lo) per 128-row chunk
    # +8 KB pad per partition row: a power-of-two row stride aliases DRAM
    # banks (measured 228 -> 384 GB/s/core on the 128 MB stream)
    pad = 8192 // _DT_SIZE[stream_dt]
    b2t = None
    if dr:
        # bias for the node-major layer-2 output (channels on the free dim)
        b2t = nc.dram_tensor("b2t", [128, C], F32, kind="ExternalInput").ap()
        # degree-normalization factored out of the adjacency so `at` can be
        # exact integer counts in fp8 (half the bytes of bf16):
        # dinvt[p, i] = dinv[128 i + p]; dinvb = dinv broadcast over channels
        dinvt = nc.dram_tensor("dinvt", [128, 8], F32, kind="ExternalInput").ap()
        dinvb = nc.dram_tensor("dinvb", [C, N_NODES], F32,
                               kind="ExternalInput").ap()
    l1w = nc.dram_tensor("l1w", [128, NK * sub * COLS + pad], stream_dt,
                         kind="ExternalInput").ap()
    l1bt = nc.dram_tensor("l1bt", [128, COLS // 128], F32, kind="ExternalInput").ap()
    l2w = nc.dram_tensor("l2w", [128, (COLS // 128) * N_OUT], F32,
                         kind="ExternalInput").ap()
    out = nc.dram_tensor("out", [1, N_OUT], F32, kind="ExternalOutput").ap()

    n_vj = COLS // 128            # 16 v1 chunks
    n_ng = COLS // 512            # 4 psum bank groups for the big matmul
    ndma = NK // cpd              # big-stream DMA count

    with tile.TileContext(nc) as tc, ExitStack() as ctx:
        const = ctx.enter_context(tc.tile_pool(name="const", bufs=1))
        small = ctx.enter_context(tc.tile_pool(name="small", bufs=1))
        wpool = ctx.enter_context(tc.tile_pool(name="wpool",
                                               bufs=12 if dr else 8))
        dpool = ctx.enter_context(tc.tile_pool(name="dpool", bufs=1, space="DRAM"))
        # dr: the 2MB adjacency + weight stream issue on the ACT hwdge queue,
        # concurrent with the small consts issuing on SP - the stream's first
        # transfer hits the DMA engines ~5us earlier.
        wq = nc.scalar if dr else nc.sync

        # ---- constant loads. The adjacency goes FIRST on the SP queue (it
        # is the biggest prefix transfer and gates the GCN); small consts
        # follow on SP; the weight stream issues concurrently on ACT.
        at_dt = F8E4 if dr else agg_dt
        at_sb = const.tile([128, 8 * N_NODES], at_dt, tag="at")
        nc.sync.dma_start(at_sb[:, :], at)
        xt_sb = const.tile([C, N_NODES], F32, tag="xt")
        nc.sync.dma_start(xt_sb[:, :], xt)
        w1_sb = const.tile([C, C], F32, tag="w1")
        nc.sync.dma_start(w1_sb[:, :], w1)
        b1_sb = const.tile([C, 1], F32, tag="b1")
        nc.sync.dma_start(b1_sb[:, :], b1)
        w2_sb = const.tile([C, C], F32, tag="w2")
        nc.sync.dma_start(w2_sb[:, :], w2)
        b2_sb = const.tile([C, 1], F32, tag="b2")
        nc.sync.dma_start(b2_sb[:, :], b2)
        l1bt_sb = const.tile([128, n_vj], F32, tag="l1bt")
        nc.sync.dma_start(l1bt_sb[:, :], l1bt)
        l2w_sb = const.tile([128, n_vj * N_OUT], F32, tag="l2w")
        nc.sync.dma_start(l2w_sb[:, :], l2w)
        if dr:
            b2t_sb = const.tile([128, C], F32, tag="b2t")
            nc.sync.dma_start(b2t_sb[:, :], b2t)
            dinvt_sb = const.tile([128, 8], F32, tag="dinvt")
            nc.sync.dma_start(dinvt_sb[:, :], dinvt)
            dinvb_sb = const.tile([C, N_NODES], F32, tag="dinvb")
            nc.sync.dma_start(dinvb_sb[:, :], dinvb)

        # ---- GCN: two layers of  hT' = relu( (AT.T-aggregated (h W)) + b )
        # h is kept transposed: [16 channels (partitions), 1024 nodes].
        def gcn_layer(h_in, w_sb, b_sb, psz, psh, zpool, hpool, li):
            # z = h @ W, built node-tile-major: z_i [128 nodes, 16]
            z_tiles = []
            for i in range(8):
                zps = psz.tile([128, C], F32, tag="zps")
                nc.tensor.matmul(zps[:, :], h_in[:, 128 * i:128 * (i + 1)],
                                 w_sb[:, :], start=True, stop=True)
                z_sb = zpool.tile([128, C], agg_dt, tag=f"z{li}_{i}")
                nc.vector.tensor_copy(z_sb[:, :], zps[:, :])
                z_tiles.append(z_sb)
            # aggregate: outT[c, d] = sum_s z[s, c] * AT[s, d]
            hps = psh.tile([C, N_NODES], F32, tag="hps")
            for i in range(8):
                for hh in range(2):
                    nc.tensor.matmul(
                        hps[:, 512 * hh:512 * (hh + 1)],
                        z_tiles[i][:, :],
                        at_sb[:, 1024 * i + 512 * hh:1024 * i + 512 * (hh + 1)],
                        start=(i == 0), stop=(i == 7),
                    )
            h_out = hpool.tile([C, N_NODES], F32, tag=f"h{li}")
            nc.scalar.activation(h_out[:, :], hps[:, :], AF.Relu, bias=b_sb[:, :])
            return h_out

        with tc.tile_pool(name="psz", bufs=2, space="PSUM") as psz, \
             tc.tile_pool(name="psh", bufs=2, space="PSUM") as psh, \
             tc.tile_pool(name="zpool", bufs=1) as zpool, \
             tc.tile_pool(name="hpool", bufs=1) as hpool:
            vcol = small.tile([128, NK], F32, tag="vcol")
            MUL = mybir.AluOpType.mult
            ADD = mybir.AluOpType.add
            if dr:
                # layer 1, channel-major out, with the degree normalization
                # applied as dinv_s on z (per-partition) and dinv_d on the
                # aggregated output (elementwise over the free dim)
                z1 = []
                for i in range(8):
                    zps = psz.tile([128, C], F32, tag="zps")
                    nc.tensor.matmul(zps[:, :], xt_sb[:, 128 * i:128 * (i + 1)],
                                     w1_sb[:, :], start=True, stop=True)
                    z_sb = zpool.tile([128, C], agg_dt, tag=f"z1_{i}")
                    nc.vector.tensor_scalar_mul(z_sb[:, :], zps[:, :],
                                                dinvt_sb[:, i:i + 1])
                    z1.append(z_sb)
                hps = psh.tile([C, N_NODES], F32, tag="hps")
                for i in range(8):
                    for hh in range(2):
                        nc.tensor.matmul(
                            hps[:, 512 * hh:512 * (hh + 1)],
                            z1[i][:, :],
                            at_sb[:, 1024 * i + 512 * hh:1024 * i + 512 * (hh + 1)],
                            start=(i == 0), stop=(i == 7))
                hmul = hpool.tile([C, N_NODES], F32, tag="hmul")
                nc.vector.tensor_mul(hmul[:, :], hps[:, :], dinvb_sb[:, :])
                h1 = hpool.tile([C, N_NODES], F32, tag="h1")
                nc.scalar.activation(h1[:, :], hmul[:, :], AF.Relu,
                                     bias=b1_sb[:, :])
            else:
                h1 = gcn_layer(xt_sb, w1_sb, b1_sb, psz, psh, zpool, hpool, 1)
            if dr:
                # ---- layer 2 with node-major output: AT-slab-stationary
                # matmuls give [128 nodes, 16 ch] tiles that are written
                # straight into vcol columns; the matching v-element order is
                # folded into the host-side L1_w row permutation, so no
                # device-side transpose/gather of v is needed at all.
                # vcol[p, 16 i + c] = v[16 (128 i + p) + c]
                z2 = []
                for i in range(8):
                    zps = psz.tile([128, C], F32, tag="zps")
                    nc.tensor.matmul(zps[:, :], h1[:, 128 * i:128 * (i + 1)],
                                     w2_sb[:, :], start=True, stop=True)
                    z_sb = zpool.tile([128, C], agg_dt, tag=f"z2_{i}")
                    nc.vector.tensor_scalar_mul(z_sb[:, :], zps[:, :],
                                                dinvt_sb[:, i:i + 1])
                    z2.append(z_sb)
                for i in range(8):
                    pd = psh.tile([128, C], F32, tag="pd")
                    for ss in range(8):
                        nc.tensor.matmul(
                            pd[:, :],
                            at_sb[:, 1024 * ss + 128 * i:1024 * ss + 128 * (i + 1)],
                            z2[ss][:, :], start=(ss == 0), stop=(ss == 7))
                    # vcol_slice = (pd * dinv_d) + b2  in one DVE op
                    nc.vector.scalar_tensor_tensor(
                        vcol[:, C * i:C * (i + 1)], pd[:, :],
                        dinvt_sb[:, i:i + 1], b2t_sb[:, :], MUL, ADD)
                nc.vector.tensor_relu(vcol[:, :], vcol[:, :])
            else:
                h2 = gcn_layer(h1, w2_sb, b2_sb, psz, psh, zpool, hpool, 2)
                # ---- vcol: v-chunks as stationary columns.
                # vcol[16a+c, k] = v[128k+16a+c] = h2[8k+a, c] = h2T[c, 8k+a]
                h2v = h2[:, :].rearrange("c (k a) -> c k a", a=8)
                for a in range(8):
                    nc.gpsimd.dma_start(vcol[16 * a:16 * (a + 1), :], h2v[:, :, a])

        if split:
            vhi = small.tile([128, NK], BF16, tag="vhi")
            nc.vector.tensor_copy(vhi[:, :], vcol[:, :])
            vhi_f = small.tile([128, NK], F32, tag="vhif")
            nc.vector.tensor_copy(vhi_f[:, :], vhi[:, :])
            vlo_f = small.tile([128, NK], F32, tag="vlof")
            nc.vector.tensor_sub(vlo_f[:, :], vcol[:, :], vhi_f[:, :])
            vlo = small.tile([128, NK], BF16, tag="vlo")
            nc.vector.tensor_copy(vlo[:, :], vlo_f[:, :])
            # passes: (stationary vec, hi/lo weight sub-chunk)
            passes = [(vhi, 0), (vlo, 0), (vhi, 1)]
        elif stream_dt == F32:
            passes = [(vcol, 0)]
        else:
            vs = small.tile([128, NK], stream_dt, tag="vs")
            nc.vector.tensor_copy(vs[:, :], vcol[:, :])
            passes = [(vs, 0)]

        # ---- big matmul: vps[0, n] = sum_k v[k] * L1[k, n]
        with tc.tile_pool(name="psv", bufs=1, space="PSUM") as psv, \
             tc.tile_pool(name="ps32", bufs=1, space="PSUM") as ps32:
            vps = psv.tile([1, COLS], F32, tag="vps")
            v1t = small.tile([128, n_vj], F32, tag="v1t")
            p32 = ps32.tile([1, N_OUT], F32, tag="p32")
            if dr:
                # Column-halved stream: half h streams all 128 k-chunks for
                # output columns [1024h, 1024h+1024), so half 0's psum drain /
                # transpose / relu / second-matmul tail runs while half 1 is
                # still streaming; only half 1's tail sits after the last DMA.
                HC = COLS // 2
                n_ht = 16                  # 1 MiB tiles per half
                cpt = NK // n_ht           # 8 chunks (4 DoubleRow pairs) /tile
                hwfree = cpt * HC
                v3 = passes[0][0][:, :].rearrange("p (s q) -> p s q", s=2)
                ident = small.tile([1, 1], F32, tag="ident")
                nc.any.memset(ident[:, :], 1.0)
                ptp = ps32.tile([128, n_vj], F32, tag="ptp")
                for rep in range(repeat):
                    for h in range(2):
                        hb = h * NK * HC
                        # split half 1's final tile so its matmuls start a
                        # sub-tile earlier after the last DMA lands
                        segs = [(t, 0, cpt) for t in range(n_ht - 1)]
                        if h == 1:
                            segs += [(n_ht - 1, 0, cpt // 2),
                                     (n_ht - 1, cpt // 2, cpt)]
                        else:
                            segs += [(n_ht - 1, 0, cpt)]
                        for t, c0, c1 in segs:
                            nch = c1 - c0
                            wt = wpool.tile([128, nch * HC], stream_dt,
                                            tag="w")
                            off = hb + hwfree * t + c0 * HC
                            wq.dma_start(wt[:, :],
                                         l1w[:, off:off + nch * HC])
                            for pp in range(nch // 2):
                                P = (cpt * t + c0) // 2 + pp
                                w3 = wt[:, 2 * pp * HC:(2 * pp + 2) * HC] \
                                    .rearrange("p (s c) -> p s c", s=2)
                                for j in range(2):
                                    nc.tensor.matmul(
                                        vps[0:1, HC * h + 512 * j:
                                            HC * h + 512 * (j + 1)],
                                        v3[:, :, P:P + 1],
                                        w3[:, :, 512 * j:512 * (j + 1)],
                                        start=(P == 0 and rep == 0),
                                        stop=(P == NK // 2 - 1
                                              and rep == repeat - 1),
                                        perf_mode=mybir.MatmulPerfMode.DoubleRow,
                                    )
                        if rep != repeat - 1:
                            continue
                        # per-half tail; half 0's overlaps half 1's stream.
                        # v1row copies go on DVE (the ACT queue is busy
                        # issuing stream DMAs in program order).
                        v1row = small.tile([1, HC], F32, tag=f"v1row{h}")
                        if h == 0:
                            nc.vector.tensor_copy(v1row[:, :],
                                                  vps[0:1, 0:HC])
                        else:
                            nc.scalar.copy(v1row[:, 0:512],
                                           vps[0:1, HC:HC + 512])
                            nc.vector.tensor_copy(v1row[:, 512:HC],
                                                  vps[0:1, HC + 512:COLS])
                        for j in range(8):
                            jj = 8 * h + j
                            nc.tensor.matmul(
                                ptp[:, jj:jj + 1],
                                v1row[0:1, 128 * j:128 * (j + 1)],
                                ident[0:1, 0:1], is_transpose=True,
                                start=True, stop=True)
                        sl = slice(8 * h, 8 * h + 8)
                        nc.vector.tensor_add(v1t[:, sl], ptp[:, sl],
                                             l1bt_sb[:, sl])
                        nc.vector.tensor_relu(v1t[:, sl], v1t[:, sl])
                        for j in range(8):
                            jj = 8 * h + j
                            nc.tensor.matmul(
                                p32[0:1, :], v1t[:, jj:jj + 1],
                                l2w_sb[:, N_OUT * jj:N_OUT * (jj + 1)],
                                start=(jj == 0), stop=(jj == n_vj - 1),
                            )
            else:
                wfree = COLS * sub * cpd     # tile free elems per DMA
                for rep in range(repeat):
                    for t in range(ndma):
                        wt = wpool.tile([128, wfree], stream_dt, tag="w")
                        wq.dma_start(wt[:, :],
                                     l1w[:, wfree * t:wfree * (t + 1)])
                        for cc in range(cpd):
                            k = cpd * t + cc
                            for j in range(n_ng):
                                for si, (vv, wi) in enumerate(passes):
                                    base = (sub * cc + wi) * 2048
                                    nc.tensor.matmul(
                                        vps[0:1, 512 * j:512 * (j + 1)],
                                        vv[:, k:k + 1],
                                        wt[:, base + 512 * j:base + 512 * (j + 1)],
                                        start=(k == 0 and si == 0 and rep == 0),
                                        stop=(k == NK - 1
                                              and si == len(passes) - 1
                                              and rep == repeat - 1),
                                    )

                # ---- tail: v1 = relu(vps + b), [128, 16] via DRAM bounce
                v1row = small.tile([1, COLS], F32, tag="v1row")
                nc.scalar.copy(v1row[:, :], vps[0:1, :])
                dscratch = dpool.tile([1, COLS], F32, tag="dscratch")
                nc.gpsimd.dma_start(dscratch[:, :], v1row[:, :])
                nc.gpsimd.dma_start(
                    v1t[:, :],
                    dscratch[:, :].rearrange("o (j p) -> p (o j)", p=128))
                nc.vector.tensor_add(v1t[:, :], v1t[:, :], l1bt_sb[:, :])
                nc.vector.tensor_relu(v1t[:, :], v1t[:, :])

                # ---- second matmul: partial[1, 32] = sum_j v1_j^T @ L2_j
                for j in range(n_vj):
                    nc.tensor.matmul(
                        p32[0:1, :], v1t[:, j:j + 1],
                        l2w_sb[:, N_OUT * j:N_OUT * (j + 1)],
                        start=(j == 0), stop=(j == n_vj - 1),
                    )
            out_sb = small.tile([1, N_OUT], F32, tag="out")
            nc.vector.tensor_copy(out_sb[:, :], p32[0:1, :])
            nc.sync.dma_start(out, out_sb[:, :])

    nc.compile()
    return nc


def _host_adjacency_parts(edge):
    """Dense integer counts AHAT[s, d] of (self-looped) edges s->d, plus the
    symmetric-normalization vector dinv = deg^-1/2."""
    src = edge[0].astype(np.int64)
    dst = edge[1].astype(np.int64)
    loop = np.arange(N_NODES, dtype=np.int64)
    s = np.concatenate([src, loop])
    d = np.concatenate([dst, loop])
    deg = np.bincount(d, minlength=N_NODES).astype(np.float32)
    dinv = np.where(deg > 0, deg, np.float32(1.0)) ** np.float32(-0.5)
    ahat = np.zeros((N_NODES, N_NODES), np.float32)
    np.add.at(ahat, (s, d), np.float32(1.0))
    return ahat, dinv


def _host_adjacency(edge):
    """Dense AT[s, d] = sum over (self-looped, deg-normalized) edges s->d."""
    ahat, dinv = _host_adjacency_parts(edge)
    return ahat * dinv[:, None] * dinv[None, :]


_NP_DT = {F32: np.float32, F32R: np.float32, BF16: ml_dtypes.bfloat16,
          F8E4: ml_dtypes.float8_e4m3}


def _prep_inputs(x, edge, W1, b1, W2, b2, L1_w, L1_b, L2_w, mode):
    stream_dt, agg_dt, cpd, split = _MODE_CFG[mode]
    np_stream = _NP_DT[stream_dt]
    np_agg = _NP_DT[agg_dt]
    fp8 = stream_dt == F8E4
    # fp8 scaled domain: h2 (=v) carries x2^4 via W2/b2, W stream x2^10,
    # so psum is x2^14; descale via L1_b x2^14 and L2_w x2^-14.
    vs = V_SCALE if fp8 else 1.0
    ws = W_SCALE if fp8 else 1.0
    zs = vs * ws

    # partition-major tiling: AT [1024,1024] -> [128, 8*1024] with
    # at_t[p, 1024*i + d] = AT[128*i + p, d]
    if fp8:
        ahat, dinv = _host_adjacency_parts(edge)
        at = ahat.astype(_NP_DT[F8E4])   # small integer counts: exact in e4m3
        dinvt = np.ascontiguousarray(dinv.reshape(8, 128).T)
        dinvb = np.ascontiguousarray(np.tile(dinv.reshape(1, N_NODES), (C, 1)))
    else:
        at = _host_adjacency(edge).astype(np_agg)
    at = np.ascontiguousarray(
        at.reshape(8, 128, N_NODES).transpose(1, 0, 2).reshape(128, 8 * N_NODES))
    xt = np.ascontiguousarray(np.asarray(x, np.float32).T)
    w1 = np.ascontiguousarray(np.asarray(W1, np.float32))
    b1v = np.asarray(b1, np.float32).reshape(C, 1).copy()
    w2 = np.ascontiguousarray(np.asarray(W2, np.float32) * vs)
    b2v = (np.asarray(b2, np.float32) * vs).reshape(C, 1).copy()
    L1_w = np.asarray(L1_w, np.float32)
    L1_b = np.asarray(L1_b, np.float32) * zs
    L2_w = np.asarray(L2_w, np.float32) * (1.0 / zs)

    in_maps = []
    for c in range(N_CORES):
        sl = slice(COLS * c, COLS * (c + 1))
        wsl = np.ascontiguousarray(L1_w[:, sl]) * ws
        pad = 8192 // np.dtype(np_stream).itemsize
        if fp8:
            # Row permutation matching the node-major vcol layout:
            # v8 column j = 16 i + c holds v elements 16*(128 i + p) + c, and
            # DMA chunk position kpos = 2 q + s streams v8 column j = s*64+q
            # (the DoubleRow slot pair for pair q is columns (q, 64+q)).
            Wr = wsl.reshape(8, 128, C, COLS)            # [i, p, c, n]
            Wr = Wr.transpose(0, 2, 1, 3).reshape(NK, 128, COLS)  # [j, p, n]
            Wr = (Wr.reshape(2, 64, 128, COLS)           # [s, q, p, n]
                  .transpose(1, 0, 2, 3).reshape(NK, 128, COLS))  # [kpos,p,n]
            # column-halved stream order: [p, (half, kpos, n)]
            body = (Wr.reshape(NK, 128, 2, COLS // 2).astype(np_stream)
                    .transpose(1, 2, 0, 3).reshape(128, NK * COLS))
        elif split:
            hi = wsl.astype(ml_dtypes.bfloat16)
            lo = (wsl - hi.astype(np.float32)).astype(ml_dtypes.bfloat16)
            # partition-major, k-major then hi/lo:
            # l1[p, (2k+s)*2048 + n] = (hi if s==0 else lo)[128k+p, n]
            body = np.empty((NK, 2, 128, COLS), ml_dtypes.bfloat16)
            body[:, 0] = hi.reshape(NK, 128, COLS)
            body[:, 1] = lo.reshape(NK, 128, COLS)
            body = body.transpose(2, 0, 1, 3).reshape(128, NK * 2 * COLS)
        else:
            # l1[p, 2048k + n] = Wslice[128k + p, n]
            body = (wsl.astype(np_stream).reshape(NK, 128, COLS)
                    .transpose(1, 0, 2).reshape(128, NK * COLS))
        l1 = np.zeros((128, body.shape[1] + pad), np_stream)
        l1[:, :body.shape[1]] = body
        l1bt = np.ascontiguousarray(L1_b[sl].reshape(COLS // 128, 128).T)
        # l2[p, 32j + n] = L2slice[128j + p, n]
        l2 = np.ascontiguousarray(
            L2_w[sl, :].reshape(COLS // 128, 128, N_OUT)
            .transpose(1, 0, 2).reshape(128, (COLS // 128) * N_OUT))
        im = dict(at=at, xt=xt, w1=w1, b1=b1v, w2=w2, b2=b2v,
                  l1w=l1, l1bt=l1bt, l2w=l2)
        if fp8:
            im["b2t"] = np.ascontiguousarray(np.tile(b2v.reshape(1, C),
                                                     (128, 1)))
            im["dinvt"] = dinvt
            im["dinvb"] = dinvb
        in_maps.append(im)
    return in_maps


def kernel(**inputs):
    global LAST_RESULT
    mode = MODE
    if mode not in _program_cache:
        _program_cache[mode] = _build(mode)
    nc = _program_cache[mode]

    in_maps = _prep_inputs(
        inputs["x"], inputs["edge"], inputs["W1"], inputs["b1"],
        inputs["W2"], inputs["b2"], inputs["L1_w"], inputs["L1_b"],
        inputs["L2_w"], mode)

    res = run_bass_kernel_spmd(
        nc, in_maps, core_ids=list(range(N_CORES)), trace=TRACE)
    LAST_RESULT = res

    partial = np.zeros(N_OUT, np.float64)
    for r in res.results:
        partial += r["out"].reshape(-1).astype(np.float64)
    logits = partial.astype(np.float32) + np.asarray(inputs["L2_b"], np.float32)
    return (1.0 / (1.0 + np.exp(-logits))).astype(np.float32)

